# revision 41
# baseline (speedup 1.0000x reference)
"""Multi-head attention (b=4, s=2048, E=1024, 16 heads, d_k=64) on 8 trn2 cores.

Sharding: core = (batch b in 0..3, head-group g in 0..1); each core handles one
batch and 8 of the 16 heads (Megatron-style column-parallel QKV + row-parallel
out-proj). Each core returns its partial out-projection [2048, 1024]; the host
sums each batch's pair of partials in fp32 (the 2-way all-reduce, done during
unsharding).

Matmul operands are fp16 (fp32 PSUM accumulation; fp16 keeps the full PE rate
of bf16 with 4x finer mantissa, and every tensor here is comfortably inside
fp16 range). Scores are computed transposed ([k, q] layout, two heads
row-packed in the PE array, both written into one 2-bank PSUM tile so a
single ACT instruction applies exp with the 1/sqrt(64) scale fused). V
carries an extra all-ones column so the attn^T @ V matmul also produces the
softmax denominator for free; the two denominator rows are matmul-broadcast
across partitions and a single fast-approx DVE reciprocal off PSUM feeds the
fused divide on eviction, which feeds the out-projection directly. Emission
order interleaves projection groups into the ACT-paced attention stream so
the PE fills exp-bound gaps.
"""

import sys

if "/opt/trn_rl_repo" not in sys.path:
    sys.path.insert(0, "/opt/trn_rl_repo")

import numpy as np

N_CORES = 8
B, S, E, H, DK = 4, 2048, 1024, 16, 64
GH = 512          # head-group width: 8 heads * 64
QB = 512          # query block (free dim of scores matmuls)
N_QB = S // QB    # 4
N_KT = S // 128   # 16 k-tiles
N_EC = E // 128   # 8 contraction chunks for projections
N_M = GH // 128   # 4 dq tiles / head pairs
DKE = DK + 1      # V head width incl the ones column

_NC = None


def build_nc():
    import concourse.tile as tile
    from concourse import bacc, mybir

    f32 = mybir.dt.float32
    bf16 = mybir.dt.float16  # fp16: same PE rate, 4x better mantissa than bf16
    Exp = mybir.ActivationFunctionType.Exp

    nc = bacc.Bacc("TRN2", target_bir_lowering=False, debug=False,
                   num_devices=N_CORES)

    xT = nc.dram_tensor("xT", [E, S], bf16, kind="ExternalInput")
    wq = nc.dram_tensor("wq", [E, GH], bf16, kind="ExternalInput")
    wk = nc.dram_tensor("wk", [E, GH], bf16, kind="ExternalInput")
    wv = nc.dram_tensor("wv", [E, GH], bf16, kind="ExternalInput")
    wo = nc.dram_tensor("wo", [GH, E], bf16, kind="ExternalInput")
    sel = nc.dram_tensor("sel", [2, 128], bf16, kind="ExternalInput")
    y = nc.dram_tensor("y", [S, E], f32, kind="ExternalOutput")

    with tile.TileContext(nc) as tc:
        with tc.tile_pool(name="persist", bufs=1) as persist, \
             tc.tile_pool(name="ps_mm", bufs=2, space="PSUM") as ps_mm, \
             tc.tile_pool(name="ps_sc", bufs=2, space="PSUM") as ps_sc, \
             tc.tile_pool(name="ps_oa", bufs=1, space="PSUM") as ps_oa, \
             tc.tile_pool(name="ps_ob", bufs=1, space="PSUM") as ps_ob:

            QT = persist.tile([128, N_M, S], bf16)   # [p, m, s]: q^T row m*128+p
            KT = persist.tile([128, N_M, S], bf16)
            # V with a ones column appended per head: [p, st, h*65+c]
            Vx = persist.tile([128, N_KT, 8 * DKE], bf16)
            nc.vector.memset(Vx[:], 1.0)
            wo_sb = persist.tile([128, N_M, E], bf16)
            # selA broadcasts denom A to partitions 0:64, selB to 64:128
            selA = persist.tile([1, 128], bf16)
            nc.sync.dma_start(selA[:], sel[0:1, :])
            selB = persist.tile([1, 128], bf16)
            nc.sync.dma_start(selB[:], sel[1:2, :])

            # ---------- inputs + projections (V, K^T first, then Q^T per
            # query block, interleaved with attention so PE fills ACT-paced
            # gaps) ----------
            with tc.tile_pool(name="xt", bufs=1) as xt_pool, \
                 tc.tile_pool(name="wchunk", bufs=4) as wch, \
                 tc.tile_pool(name="wvchunk", bufs=1) as wvch, \
                 tc.tile_pool(name="expp", bufs=14) as expp, \
                 tc.tile_pool(name="d2p", bufs=4) as d2p, \
                 tc.tile_pool(name="recipp", bufs=3) as recipp, \
                 tc.tile_pool(name="outTp", bufs=2) as outTp, \
                 tc.tile_pool(name="finp", bufs=4) as finp:
                # DMAs in need-order: x^T, first K/Q weight slabs (scores for
                # pair 0 are the critical path to the first exp), wv, the
                # remaining slabs, wo last.
                xt = xt_pool.tile([128, N_EC, S], bf16)
                for ec in range(N_EC):
                    nc.sync.dma_start(xt[:, ec, :], xT[ec * 128:(ec + 1) * 128, :])

                qchunks, kchunks = {}, {}

                def load_slab(dst, w_dram, wname, m):
                    slab = wch.tile([128, N_EC, 128], bf16, name=f"w{wname}s")
                    nc.sync.dma_start(
                        slab[:],
                        w_dram[:, m * 128:(m + 1) * 128].rearrange(
                            "(eo p) d -> p eo d", p=128))
                    for ec in range(N_EC):
                        dst[(m, ec)] = slab[:, ec, :]

                wv_sb = wvch.tile([128, N_EC, GH], bf16, name="wvs")
                nc.sync.dma_start(wv_sb[:],
                                  wv.rearrange("(eo p) d -> p eo d", p=128))
                vchunks = [wv_sb[:, ec, :] for ec in range(N_EC)]
                for m in range(N_M):
                    load_slab(kchunks, wk, "k", m)
                    load_slab(qchunks, wq, "q", m)
                for fo in range(N_M):
                    nc.sync.dma_start(wo_sb[:, fo, :], wo[fo * 128:(fo + 1) * 128, :])

                def emit_v():
                    for st in range(N_KT):
                        ps = ps_mm.tile([128, GH], f32, name="mm")
                        for ec in range(N_EC):
                            nc.tensor.matmul(
                                ps[:], xt[:, ec, st * 128:(st + 1) * 128],
                                vchunks[ec][:],
                                start=(ec == 0), stop=(ec == N_EC - 1))
                        # scatter into the ones-padded layout: [128, 8, 64]
                        vdst = Vx[:, st, :].rearrange("p (h c) -> p h c", c=DKE)
                        nc.vector.tensor_copy(
                            vdst[:, :, 0:DK],
                            ps[:].rearrange("p (h c) -> p h c", c=DK))

                def emit_kt(m):
                    for sb in range(N_QB):
                        ps = ps_mm.tile([128, QB], f32, name="mm")
                        for ec in range(N_EC):
                            nc.tensor.matmul(
                                ps[:], kchunks[(m, ec)][:],
                                xt[:, ec, sb * QB:(sb + 1) * QB],
                                start=(ec == 0), stop=(ec == N_EC - 1))
                        nc.vector.tensor_copy(
                            KT[:, m, sb * QB:(sb + 1) * QB], ps[:])

                def emit_qt(sb, m):
                    ssl = slice(sb * QB, (sb + 1) * QB)
                    ps = ps_mm.tile([128, QB], f32, name="mm")
                    for ec in range(N_EC):
                        nc.tensor.matmul(
                            ps[:], qchunks[(m, ec)][:], xt[:, ec, ssl],
                            start=(ec == 0), stop=(ec == N_EC - 1))
                    nc.vector.tensor_copy(QT[:, m, ssl], ps[:])

                outTs = {}

                def emit_pair(qb, j):
                    qsl = slice(qb * QB, (qb + 1) * QB)
                    outT = outTs[qb]
                    oTA = ps_oa.tile([DKE, QB], f32, name="ps_oTA")
                    oTB = ps_ob.tile([DKE, QB], f32, name="ps_oTB")
                    for kt in range(N_KT):
                        ksl = slice(kt * 128, (kt + 1) * 128)
                        # scores^T for both heads, row-packed, one psum tile
                        psS = ps_sc.tile([128, 2, QB], f32, name="psS")
                        nc.tensor.matmul(psS[:, 0, :], KT[0:64, j, ksl],
                                         QT[0:64, j, qsl])
                        nc.tensor.matmul(psS[:, 1, :], KT[64:128, j, ksl],
                                         QT[64:128, j, qsl])
                        # exp(score/8) for both heads in one ACT
                        eAB = expp.tile([128, 2, QB], bf16, name="eAB")
                        nc.scalar.activation(eAB[:], psS[:], Exp, scale=0.125)
                        # attn^T @ [V | 1] -> out rows 0:64, denom row 64
                        nc.tensor.matmul(
                            oTA[:, :],
                            Vx[:, kt, (2 * j) * DKE:(2 * j + 1) * DKE],
                            eAB[:, 0, :], start=(kt == 0),
                            stop=(kt == N_KT - 1))
                        nc.tensor.matmul(
                            oTB[:, :],
                            Vx[:, kt, (2 * j + 1) * DKE:(2 * j + 2) * DKE],
                            eAB[:, 1, :], start=(kt == 0),
                            stop=(kt == N_KT - 1))
                    # softmax 1/denom: matmul-broadcast the two denom rows
                    # (fp16) across partitions, then one fast-approx DVE
                    # reciprocal off PSUM (~18-bit) doubling as the eviction.
                    dA = d2p.tile([1, QB], bf16, name="dA")
                    nc.vector.tensor_copy(dA[:], oTA[DK:DKE, :])
                    dB = d2p.tile([1, QB], bf16, name="dB")
                    nc.vector.tensor_copy(dB[:], oTB[DK:DKE, :])
                    bcD = ps_mm.tile([128, QB], f32, name="mm")
                    nc.tensor.matmul(bcD[:], selA[:], dA[:],
                                     start=True, stop=False)
                    nc.tensor.matmul(bcD[:], selB[:], dB[:],
                                     start=False, stop=True)
                    recipB = recipp.tile([128, QB], f32, name="recipB")
                    nc.vector.reciprocal_approx_fast(recipB[:], bcD[:])
                    # evict attn output with the softmax divide fused
                    nc.vector.tensor_mul(outT[0:64, j, :], oTA[0:DK, :],
                                         recipB[0:64, :])
                    nc.vector.tensor_mul(outT[64:128, j, :], oTB[0:DK, :],
                                         recipB[64:128, :])

                def emit_outproj(qb):
                    # y[s, e] = outT^T @ wo; eb inner so each outT chunk is
                    # loaded as weights once
                    outT = outTs.pop(qb)
                    for st in range(QB // 128):
                        row0 = qb * QB + st * 128
                        pss = [ps_mm.tile([128, QB], f32, name="mm")
                               for _ in range(E // QB)]
                        for jj in range(N_M):
                            for eb in range(E // QB):
                                nc.tensor.matmul(
                                    pss[eb][:],
                                    outT[:, jj, st * 128:(st + 1) * 128],
                                    wo_sb[:, jj, eb * QB:(eb + 1) * QB],
                                    start=(jj == 0), stop=(jj == N_M - 1))
                        for eb in range(E // QB):
                            fo = finp.tile([128, QB], f32, name="fo")
                            nc.vector.tensor_copy(fo[:], pss[eb][:])
                            nc.sync.dma_start(
                                y[row0:row0 + 128, eb * QB:(eb + 1) * QB], fo[:])

                # Interleave: emit each attention pair first (so its scores
                # keep ACT fed), then the projection groups that unblock the
                # NEXT pair / next query block -- those run on the PE during
                # the ACT-paced stretch of the current pair. NOTE: Tile
                # dependencies follow emission order, so V and each pair's
                # K^T/Q^T groups must be emitted before the pair that reads
                # them.
                emit_v()
                outTs[0] = outTp.tile([128, N_M, QB], bf16, name="outT")
                emit_kt(0)
                emit_qt(0, 0)
                for j in range(N_M):
                    emit_pair(0, j)
                    if j + 1 < N_M:
                        emit_kt(j + 1)
                        emit_qt(0, j + 1)
                    emit_qt(1, j)
                for qb in range(1, N_QB):
                    outTs[qb] = outTp.tile([128, N_M, QB], bf16, name="outT")
                    for j in range(N_M):
                        emit_pair(qb, j)
                        if qb + 1 < N_QB:
                            emit_qt(qb + 1, j)
                        if j == 0:
                            emit_outproj(qb - 1)
                emit_outproj(N_QB - 1)

    nc.finalize()
    return nc


def get_nc():
    global _NC
    if _NC is None:
        _NC = build_nc()
    return _NC


def _bf(a):
    return np.ascontiguousarray(a).astype(np.float16)


def make_in_maps(x, Wq, Wk, Wv, Wo):
    selmat = np.zeros((2, 128), np.float16)
    selmat[0, 0:64] = 1.0
    selmat[1, 64:128] = 1.0
    in_maps = []
    for b in range(B):
        xTb = _bf(np.asarray(x[b], np.float32).T)
        for g in range(2):
            sl = slice(g * GH, (g + 1) * GH)
            in_maps.append(dict(
                xT=xTb,
                wq=_bf(np.asarray(Wq, np.float32)[sl, :].T),
                wk=_bf(np.asarray(Wk, np.float32)[sl, :].T),
                wv=_bf(np.asarray(Wv, np.float32)[sl, :].T),
                wo=_bf(np.asarray(Wo, np.float32)[:, sl].T),
                sel=selmat,
            ))
    return in_maps


def combine(results):
    out = np.empty((B, S, E), np.float32)
    for b in range(B):
        out[b] = results[2 * b]["y"] + results[2 * b + 1]["y"]
    return out


def kernel(x, Wq, Wk, Wv, Wo):
    from concourse.bass_utils import run_bass_kernel_spmd
    res = run_bass_kernel_spmd(
        get_nc(), make_in_maps(x, Wq, Wk, Wv, Wo),
        core_ids=list(range(N_CORES)))
    return combine(res.results)


# revision 43
# speedup vs baseline: 1.0085x; 1.0085x over previous
"""Multi-head attention (b=4, s=2048, E=1024, 16 heads, d_k=64) on 8 trn2 cores.

Sharding: core = (batch b in 0..3, head-group g in 0..1); each core handles one
batch and 8 of the 16 heads (Megatron-style column-parallel QKV + row-parallel
out-proj). Each core returns its partial out-projection [2048, 1024]; the host
sums each batch's pair of partials in fp32 (the 2-way all-reduce, done during
unsharding).

Matmul operands are fp16 (fp32 PSUM accumulation; fp16 keeps the full PE rate
of bf16 with 4x finer mantissa, and every tensor here is comfortably inside
fp16 range). Scores are computed transposed ([k, q] layout, two heads
row-packed in the PE array, both written into one 2-bank PSUM tile so a
single ACT instruction applies exp with the 1/sqrt(64) scale fused). V
carries an extra all-ones column so the attn^T @ V matmul also produces the
softmax denominator for free; the two denominator rows are matmul-broadcast
across partitions and a single fast-approx DVE reciprocal off PSUM feeds the
fused divide on eviction, which feeds the out-projection directly. Emission
order interleaves projection groups into the ACT-paced attention stream so
the PE fills exp-bound gaps.
"""

import sys

if "/opt/trn_rl_repo" not in sys.path:
    sys.path.insert(0, "/opt/trn_rl_repo")

import numpy as np

N_CORES = 8
B, S, E, H, DK = 4, 2048, 1024, 16, 64
GH = 512          # head-group width: 8 heads * 64
QB = 512          # query block (free dim of scores matmuls)
N_QB = S // QB    # 4
N_KT = S // 128   # 16 k-tiles
N_EC = E // 128   # 8 contraction chunks for projections
N_M = GH // 128   # 4 dq tiles / head pairs
DKE = DK + 1      # V head width incl the ones column

_NC = None


def build_nc():
    import concourse.tile as tile
    from concourse import bacc, mybir

    f32 = mybir.dt.float32
    bf16 = mybir.dt.float16  # fp16: same PE rate, 4x better mantissa than bf16
    Exp = mybir.ActivationFunctionType.Exp

    nc = bacc.Bacc("TRN2", target_bir_lowering=False, debug=False,
                   num_devices=N_CORES)

    xT = nc.dram_tensor("xT", [E, S], bf16, kind="ExternalInput")
    wq = nc.dram_tensor("wq", [E, GH], bf16, kind="ExternalInput")
    wk = nc.dram_tensor("wk", [E, GH], bf16, kind="ExternalInput")
    wv = nc.dram_tensor("wv", [E, GH], bf16, kind="ExternalInput")
    wo = nc.dram_tensor("wo", [GH, E], bf16, kind="ExternalInput")
    sel = nc.dram_tensor("sel", [2, 128], bf16, kind="ExternalInput")
    y = nc.dram_tensor("y", [S, E], f32, kind="ExternalOutput")

    with tile.TileContext(nc) as tc:
        with tc.tile_pool(name="persist", bufs=1) as persist, \
             tc.tile_pool(name="ps_mm", bufs=2, space="PSUM") as ps_mm, \
             tc.tile_pool(name="ps_sc", bufs=2, space="PSUM") as ps_sc, \
             tc.tile_pool(name="ps_oa", bufs=1, space="PSUM") as ps_oa, \
             tc.tile_pool(name="ps_ob", bufs=1, space="PSUM") as ps_ob:

            QT = persist.tile([128, N_M, S], bf16)   # [p, m, s]: q^T row m*128+p
            KT = persist.tile([128, N_M, S], bf16)
            # V with a ones column appended per head: [p, st, h*65+c]
            Vx = persist.tile([128, N_KT, 8 * DKE], bf16)
            nc.vector.memset(Vx[:], 1.0)
            wo_sb = persist.tile([128, N_M, E], bf16)
            # selA broadcasts denom A to partitions 0:64, selB to 64:128
            selA = persist.tile([1, 128], bf16)
            nc.sync.dma_start(selA[:], sel[0:1, :])
            selB = persist.tile([1, 128], bf16)
            nc.sync.dma_start(selB[:], sel[1:2, :])

            # ---------- inputs + projections (V, K^T first, then Q^T per
            # query block, interleaved with attention so PE fills ACT-paced
            # gaps) ----------
            with tc.tile_pool(name="xt", bufs=1) as xt_pool, \
                 tc.tile_pool(name="wchunk", bufs=4) as wch, \
                 tc.tile_pool(name="wvchunk", bufs=1) as wvch, \
                 tc.tile_pool(name="expp", bufs=14) as expp, \
                 tc.tile_pool(name="d2p", bufs=4) as d2p, \
                 tc.tile_pool(name="recipp", bufs=3) as recipp, \
                 tc.tile_pool(name="outTp", bufs=2) as outTp, \
                 tc.tile_pool(name="finp", bufs=4) as finp:
                # DMAs in need-order: x^T, first K/Q weight slabs (scores for
                # pair 0 are the critical path to the first exp), wv, the
                # remaining slabs, wo last.
                xt = xt_pool.tile([128, N_EC, S], bf16)
                nc.sync.dma_start(xt[:, 0, :], xT[0:128, :])

                qchunks, kchunks = {}, {}

                def load_slab(dst, w_dram, wname, m):
                    slab = wch.tile([128, N_EC, 128], bf16, name=f"w{wname}s")
                    nc.sync.dma_start(
                        slab[:],
                        w_dram[:, m * 128:(m + 1) * 128].rearrange(
                            "(eo p) d -> p eo d", p=128))
                    for ec in range(N_EC):
                        dst[(m, ec)] = slab[:, ec, :]

                wv_sb = wvch.tile([128, N_EC, GH], bf16, name="wvs")
                nc.sync.dma_start(wv_sb[:],
                                  wv.rearrange("(eo p) d -> p eo d", p=128))
                vchunks = [wv_sb[:, ec, :] for ec in range(N_EC)]
                for ec in range(1, N_EC):
                    nc.sync.dma_start(xt[:, ec, :], xT[ec * 128:(ec + 1) * 128, :])
                for m in range(N_M):
                    load_slab(kchunks, wk, "k", m)
                    load_slab(qchunks, wq, "q", m)
                for fo in range(N_M):
                    nc.sync.dma_start(wo_sb[:, fo, :], wo[fo * 128:(fo + 1) * 128, :])

                def emit_v():
                    for st in range(N_KT):
                        ps = ps_mm.tile([128, GH], f32, name="mm")
                        for ec in range(N_EC):
                            nc.tensor.matmul(
                                ps[:], xt[:, ec, st * 128:(st + 1) * 128],
                                vchunks[ec][:],
                                start=(ec == 0), stop=(ec == N_EC - 1))
                        # scatter into the ones-padded layout: [128, 8, 64]
                        vdst = Vx[:, st, :].rearrange("p (h c) -> p h c", c=DKE)
                        nc.vector.tensor_copy(
                            vdst[:, :, 0:DK],
                            ps[:].rearrange("p (h c) -> p h c", c=DK))

                def emit_kt(m):
                    for sb in range(N_QB):
                        ps = ps_mm.tile([128, QB], f32, name="mm")
                        for ec in range(N_EC):
                            nc.tensor.matmul(
                                ps[:], kchunks[(m, ec)][:],
                                xt[:, ec, sb * QB:(sb + 1) * QB],
                                start=(ec == 0), stop=(ec == N_EC - 1))
                        nc.vector.tensor_copy(
                            KT[:, m, sb * QB:(sb + 1) * QB], ps[:])

                def emit_qt(sb, m):
                    ssl = slice(sb * QB, (sb + 1) * QB)
                    ps = ps_mm.tile([128, QB], f32, name="mm")
                    for ec in range(N_EC):
                        nc.tensor.matmul(
                            ps[:], qchunks[(m, ec)][:], xt[:, ec, ssl],
                            start=(ec == 0), stop=(ec == N_EC - 1))
                    nc.vector.tensor_copy(QT[:, m, ssl], ps[:])

                outTs = {}

                def emit_pair(qb, j):
                    qsl = slice(qb * QB, (qb + 1) * QB)
                    outT = outTs[qb]
                    oTA = ps_oa.tile([DKE, QB], f32, name="ps_oTA")
                    oTB = ps_ob.tile([DKE, QB], f32, name="ps_oTB")
                    for kt in range(N_KT):
                        ksl = slice(kt * 128, (kt + 1) * 128)
                        # scores^T for both heads, row-packed, one psum tile
                        psS = ps_sc.tile([128, 2, QB], f32, name="psS")
                        nc.tensor.matmul(psS[:, 0, :], KT[0:64, j, ksl],
                                         QT[0:64, j, qsl])
                        nc.tensor.matmul(psS[:, 1, :], KT[64:128, j, ksl],
                                         QT[64:128, j, qsl])
                        # exp(score/8) for both heads in one ACT
                        eAB = expp.tile([128, 2, QB], bf16, name="eAB")
                        nc.scalar.activation(eAB[:], psS[:], Exp, scale=0.125)
                        # attn^T @ [V | 1] -> out rows 0:64, denom row 64
                        nc.tensor.matmul(
                            oTA[:, :],
                            Vx[:, kt, (2 * j) * DKE:(2 * j + 1) * DKE],
                            eAB[:, 0, :], start=(kt == 0),
                            stop=(kt == N_KT - 1))
                        nc.tensor.matmul(
                            oTB[:, :],
                            Vx[:, kt, (2 * j + 1) * DKE:(2 * j + 2) * DKE],
                            eAB[:, 1, :], start=(kt == 0),
                            stop=(kt == N_KT - 1))
                    # softmax 1/denom: matmul-broadcast the two denom rows
                    # (fp16) across partitions, then one fast-approx DVE
                    # reciprocal off PSUM (~18-bit) doubling as the eviction.
                    dA = d2p.tile([1, QB], bf16, name="dA")
                    nc.vector.tensor_copy(dA[:], oTA[DK:DKE, :])
                    dB = d2p.tile([1, QB], bf16, name="dB")
                    nc.vector.tensor_copy(dB[:], oTB[DK:DKE, :])
                    bcD = ps_mm.tile([128, QB], f32, name="mm")
                    nc.tensor.matmul(bcD[:], selA[:], dA[:],
                                     start=True, stop=False)
                    nc.tensor.matmul(bcD[:], selB[:], dB[:],
                                     start=False, stop=True)
                    recipB = recipp.tile([128, QB], f32, name="recipB")
                    nc.vector.reciprocal_approx_fast(recipB[:], bcD[:])
                    # evict attn output with the softmax divide fused
                    nc.vector.tensor_mul(outT[0:64, j, :], oTA[0:DK, :],
                                         recipB[0:64, :])
                    nc.vector.tensor_mul(outT[64:128, j, :], oTB[0:DK, :],
                                         recipB[64:128, :])

                def emit_outproj(qb):
                    # y[s, e] = outT^T @ wo; eb inner so each outT chunk is
                    # loaded as weights once
                    outT = outTs.pop(qb)
                    for st in range(QB // 128):
                        row0 = qb * QB + st * 128
                        pss = [ps_mm.tile([128, QB], f32, name="mm")
                               for _ in range(E // QB)]
                        for jj in range(N_M):
                            for eb in range(E // QB):
                                nc.tensor.matmul(
                                    pss[eb][:],
                                    outT[:, jj, st * 128:(st + 1) * 128],
                                    wo_sb[:, jj, eb * QB:(eb + 1) * QB],
                                    start=(jj == 0), stop=(jj == N_M - 1))
                        for eb in range(E // QB):
                            fo = finp.tile([128, QB], f32, name="fo")
                            nc.vector.tensor_copy(fo[:], pss[eb][:])
                            nc.sync.dma_start(
                                y[row0:row0 + 128, eb * QB:(eb + 1) * QB], fo[:])

                # Interleave: emit each attention pair first (so its scores
                # keep ACT fed), then the projection groups that unblock the
                # NEXT pair / next query block -- those run on the PE during
                # the ACT-paced stretch of the current pair. NOTE: Tile
                # dependencies follow emission order, so V and each pair's
                # K^T/Q^T groups must be emitted before the pair that reads
                # them.
                emit_v()
                outTs[0] = outTp.tile([128, N_M, QB], bf16, name="outT")
                emit_kt(0)
                emit_qt(0, 0)
                for j in range(N_M):
                    emit_pair(0, j)
                    if j + 1 < N_M:
                        emit_kt(j + 1)
                        emit_qt(0, j + 1)
                    emit_qt(1, j)
                for qb in range(1, N_QB):
                    outTs[qb] = outTp.tile([128, N_M, QB], bf16, name="outT")
                    for j in range(N_M):
                        emit_pair(qb, j)
                        if qb + 1 < N_QB:
                            emit_qt(qb + 1, j)
                        if j == 0:
                            emit_outproj(qb - 1)
                emit_outproj(N_QB - 1)

    nc.finalize()
    return nc


def get_nc():
    global _NC
    if _NC is None:
        _NC = build_nc()
    return _NC


def _bf(a):
    return np.ascontiguousarray(a).astype(np.float16)


def make_in_maps(x, Wq, Wk, Wv, Wo):
    selmat = np.zeros((2, 128), np.float16)
    selmat[0, 0:64] = 1.0
    selmat[1, 64:128] = 1.0
    in_maps = []
    for b in range(B):
        xTb = _bf(np.asarray(x[b], np.float32).T)
        for g in range(2):
            sl = slice(g * GH, (g + 1) * GH)
            in_maps.append(dict(
                xT=xTb,
                wq=_bf(np.asarray(Wq, np.float32)[sl, :].T),
                wk=_bf(np.asarray(Wk, np.float32)[sl, :].T),
                wv=_bf(np.asarray(Wv, np.float32)[sl, :].T),
                wo=_bf(np.asarray(Wo, np.float32)[:, sl].T),
                sel=selmat,
            ))
    return in_maps


def combine(results):
    out = np.empty((B, S, E), np.float32)
    for b in range(B):
        out[b] = results[2 * b]["y"] + results[2 * b + 1]["y"]
    return out


def kernel(x, Wq, Wk, Wv, Wo):
    from concourse.bass_utils import run_bass_kernel_spmd
    res = run_bass_kernel_spmd(
        get_nc(), make_in_maps(x, Wq, Wk, Wv, Wo),
        core_ids=list(range(N_CORES)))
    return combine(res.results)
